# revision 1
# baseline (speedup 1.0000x reference)
import sys
sys.path.insert(0, '/opt/trn_rl_repo')
import math
import numpy as np
import ml_dtypes

import concourse.bass as bass
import concourse.tile as tile
from concourse import bacc, mybir
from concourse.bass_utils import run_bass_kernel_spmd
from concourse.masks import make_identity

DIM = 2048
NUM_HEADS = 32
NUM_KV_HEADS = 8
HD = 64
BSZ, SEQ = 2, 2048
THRESHOLD = 0.05
S = SEQ                      # seq per core (batch-sharded)
HPC = 8                      # q heads per core
KVPC = 2                     # kv heads per core
NPAIR = 4                    # head pairs per core
SB = 512                     # s-block
NSB = S // SB                # 4
NDC = DIM // 128             # 16 contraction chunks
NQT = S // 128               # 16 q tiles
NQB = S // SB                # 4 q blocks

f32 = mybir.dt.float32
f32r = mybir.dt.float32r
bf16 = mybir.dt.bfloat16
bf = ml_dtypes.bfloat16
EXP = mybir.ActivationFunctionType.Exp
AX = mybir.AxisListType.X
MAX = mybir.AluOpType.max
MIN = mybir.AluOpType.min


def _ternarize(w):
    w = w.astype(np.float64)
    scale = max(np.abs(w).mean(), 1e-6)
    return np.where(w > THRESHOLD * scale, 1.0,
                    np.where(w < -THRESHOLD * scale, -1.0, 0.0))


def _split_hi_lo(x32):
    hi = x32.astype(bf)
    lo = (x32 - hi.astype(np.float32)).astype(bf)
    return hi, lo


DEBUG = False


def build_program():
    nc = bacc.Bacc(None, target_bir_lowering=False, debug=False)

    def din(name, shape, dt):
        return nc.dram_tensor(name, list(shape), dt, kind="ExternalInput").ap()

    xhi_d = din("xhi", (DIM, S), bf16)       # x[b].T hi
    xlo_d = din("xlo", (DIM, S), bf16)       # x[b].T lo
    wq_d = din("wq", (DIM, 512), bf16)       # ternary(wq).T/8 cols of 8 heads
    wk_d = din("wk", (DIM, 128), bf16)       # ternary(wk).T cols [k0|k1]
    wv_d = din("wv", (DIM, 128), bf16)       # ternary(wv).T cols [v0|v1]
    wo_d = din("wo", (512, DIM), f32r)       # ternary(wo).T rows = core feats
    tri_d = din("tri", (128, 128), bf16)     # lower-tri 0/1 mask
    oT_d = nc.dram_tensor("oT", [DIM, S], f32, kind="ExternalOutput").ap()
    dbg = {}
    if DEBUG:
        for nm, shape in [("d_qthi", (128, S)), ("d_qtlo", (128, S)),
                          ("d_kkhi", (128, S)), ("d_kklo", (128, S)),
                          ("d_va", (128, NDC * 65)), ("d_ot", (128, S)),
                          ("d_p", (128, 4 * S)), ("d_pt", (128, NDC * SB))]:
            dbg[nm] = nc.dram_tensor(nm, list(shape), f32, kind="ExternalOutput").ap()
    scr_d = nc.dram_tensor("scr", [HPC, NQB, SB], f32).ap()  # recip rows

    with tile.TileContext(nc) as tc:
        # ---------------- persistent tiles ----------------
        with tc.tile_pool(name="persist", bufs=1) as pp:
            wq_sb = pp.tile([128, NDC, 512], bf16)
            wk_sb = pp.tile([128, NDC, 128], bf16)
            wv_sb = pp.tile([128, NDC, 128], bf16)
            tri = pp.tile([128, 128], bf16)
            nc.sync.dma_start(tri[:], tri_d[:])
            identb = pp.tile([128, 128], bf16)
            make_identity(nc, identb[:])

            # projection results
            qt_hi = [pp.tile([128, S], bf16, tag=f"qth{m}", name=f"qth{m}") for m in range(NPAIR)]
            qt_lo = [pp.tile([128, S], bf16, tag=f"qtl{m}", name=f"qtl{m}") for m in range(NPAIR)]
            kk_hi = [pp.tile([128, S], bf16, tag=f"kkh{v}", name=f"kkh{v}") for v in range(KVPC)]
            kk_lo = [pp.tile([128, S], bf16, tag=f"kkl{v}", name=f"kkl{v}") for v in range(KVPC)]
            va = [pp.tile([128, NDC, 65], bf16, tag=f"va{v}", name=f"va{v}") for v in range(KVPC)]
            ot = [pp.tile([128, S], f32r, tag=f"ot{m}", name=f"ot{m}") for m in range(NPAIR)]
            for v in range(KVPC):
                nc.vector.memset(va[v][:, :, 64:65], 1.0)

            # ---------------- phase 1: projections ----------------
            with tc.tile_pool(name="xp", bufs=4) as xp, \
                 tc.tile_pool(name="evac", bufs=2) as ev, \
                 tc.tile_pool(name="psp", bufs=1, space="PSUM") as psp:
                for sb_i in range(NSB):
                    ssl = bass.ts(sb_i, SB)
                    ps_q = [psp.tile([128, SB], f32, tag=f"psq{m}", name=f"psq{m}") for m in range(NPAIR)]
                    ps_k = psp.tile([128, SB], f32, tag="psk")
                    ps_v = psp.tile([128, SB], f32, tag="psv")
                    for dc in range(NDC):
                        xhi = xp.tile([128, SB], bf16, tag="xhi")
                        xlo = xp.tile([128, SB], bf16, tag="xlo")
                        nc.sync.dma_start(xhi[:], xhi_d[dc * 128:(dc + 1) * 128, ssl])
                        nc.sync.dma_start(xlo[:], xlo_d[dc * 128:(dc + 1) * 128, ssl])
                        if sb_i == 0:
                            nc.sync.dma_start(wq_sb[:, dc, :], wq_d[dc * 128:(dc + 1) * 128, :])
                            nc.sync.dma_start(wk_sb[:, dc, :], wk_d[dc * 128:(dc + 1) * 128, :])
                            nc.sync.dma_start(wv_sb[:, dc, :], wv_d[dc * 128:(dc + 1) * 128, :])
                        st = (dc == 0)
                        sp = (dc == NDC - 1)
                        for m in range(NPAIR):
                            wsl = wq_sb[:, dc, bass.ts(m, 128)]
                            nc.tensor.matmul(ps_q[m][:], wsl, xhi[:], start=st, stop=False)
                            nc.tensor.matmul(ps_q[m][:], wsl, xlo[:], start=False, stop=sp)
                        nc.tensor.matmul(ps_k[:], wk_sb[:, dc, :], xhi[:], start=st, stop=False)
                        nc.tensor.matmul(ps_k[:], wk_sb[:, dc, :], xlo[:], start=False, stop=sp)
                        for j in range(4):
                            nc.tensor.matmul(ps_v[:, bass.ts(j, 128)],
                                             xhi[:, bass.ts(j, 128)],
                                             wv_sb[:, dc, :],
                                             start=(st and j == 0), stop=sp)
                    # evacuate Q (hi/lo split)
                    for m in range(NPAIR):
                        nc.scalar.copy(qt_hi[m][:, ssl], ps_q[m][:])
                        nc.vector.tensor_sub(qt_lo[m][:, ssl], ps_q[m][:], qt_hi[m][:, ssl])
                    # evacuate K (hi/lo + duplicate rows for row-packing)
                    k_hi = ev.tile([128, SB], bf16, tag="khi")
                    k_lo = ev.tile([128, SB], bf16, tag="klo")
                    nc.scalar.copy(k_hi[:], ps_k[:])
                    nc.vector.tensor_sub(k_lo[:], ps_k[:], k_hi[:])
                    for v in range(KVPC):
                        vs = bass.ds(v * 64, 64)
                        nc.sync.dma_start(kk_hi[v][0:64, ssl], k_hi[vs, :])
                        nc.sync.dma_start(kk_hi[v][64:128, ssl], k_hi[vs, :])
                        nc.sync.dma_start(kk_lo[v][0:64, ssl], k_lo[vs, :])
                        nc.sync.dma_start(kk_lo[v][64:128, ssl], k_lo[vs, :])
                    for j in range(4):
                        ch = sb_i * 4 + j
                        nc.scalar.copy(va[0][:, ch, 0:64], ps_v[:, bass.ds(j * 128, 64)])
                        nc.scalar.copy(va[1][:, ch, 0:64], ps_v[:, bass.ds(j * 128 + 64, 64)])

            # ---------------- phase 2: attention ----------------
            with tc.tile_pool(name="att", bufs=1) as ap, \
                 tc.tile_pool(name="stat", bufs=4) as stp, \
                 tc.tile_pool(name="ps2", bufs=1, space="PSUM") as ps2:
                p_t = [ap.tile([128, 4, S], bf16, tag=f"p{h}", name=f"p{h}") for h in range(2)]
                pt_t = [ap.tile([128, NDC, SB], bf16, tag=f"pt{h}", name=f"pt{h}") for h in range(2)]
                for hp in range(NPAIR):
                    kv = hp // 2
                    for qb in range(NQB):
                        nch = 4 * (qb + 1)
                        # -- scores + exp per q tile --
                        for j in range(4):
                            qi = qb * 4 + j
                            nk = qi // 4 + 1
                            qsl = bass.ts(qi, 128)
                            nmx = [stp.tile([128, 4], f32, tag=f"nmx{h}", name=f"nmx{h}") for h in range(2)]
                            for kb in range(nk):
                                kw = 512 if kb < nk - 1 else 128 * (qi % 4 + 1)
                                ksl = bass.ds(kb * 512, kw)
                                s0 = [ps2.tile([128, 512], f32, tag=f"s0{h}", bufs=1, name=f"s0{h}") for h in range(2)]
                                for h in range(2):
                                    nc.tensor.matmul(
                                        s0[h][:, 0:kw],
                                        qt_hi[hp][bass.ds(h * 64, 64), qsl],
                                        kk_hi[kv][bass.ds(h * 64, 64), ksl],
                                        start=True, stop=True,
                                        tile_position=(h * 64, 0))
                                    if kb == nk - 1:
                                        nc.vector.tensor_add(
                                            s0[h][:, kw - 128:kw],
                                            s0[h][:, kw - 128:kw], tri[:])
                                    nc.vector.tensor_reduce(
                                        nmx[h][:, kb:kb + 1], s0[h][:, 0:kw],
                                        AX, MAX, negate=True)
                            negmax = [stp.tile([128, 1], f32, tag=f"ngm{h}", name=f"ngm{h}") for h in range(2)]
                            for h in range(2):
                                nc.vector.tensor_reduce(
                                    negmax[h][:], nmx[h][:, 0:nk], AX, MIN)
                            # accurate scores (hh + hl + lh) then exp
                            for kb in range(nk):
                                kw = 512 if kb < nk - 1 else 128 * (qi % 4 + 1)
                                ksl = bass.ds(kb * 512, kw)
                                sa = [ps2.tile([128, 512], f32, tag=f"sa{h}", name=f"sa{h}") for h in range(2)]
                                for h in range(2):
                                    hs = bass.ds(h * 64, 64)
                                    tp = (h * 64, 0)
                                    nc.tensor.matmul(sa[h][:, 0:kw], qt_hi[hp][hs, qsl],
                                                     kk_hi[kv][hs, ksl], start=True,
                                                     stop=False, tile_position=tp)
                                    nc.tensor.matmul(sa[h][:, 0:kw], qt_hi[hp][hs, qsl],
                                                     kk_lo[kv][hs, ksl], start=False,
                                                     stop=False, tile_position=tp)
                                    nc.tensor.matmul(sa[h][:, 0:kw], qt_lo[hp][hs, qsl],
                                                     kk_hi[kv][hs, ksl], start=False,
                                                     stop=True, tile_position=tp)
                                    if kb == nk - 1:
                                        nc.vector.tensor_add(
                                            sa[h][:, kw - 128:kw],
                                            sa[h][:, kw - 128:kw], tri[:])
                                    nc.scalar.activation(
                                        p_t[h][:, j, ksl], sa[h][:, 0:kw], EXP,
                                        bias=negmax[h][:], scale=1.0)
                        # -- transpose P, zero invalid chunks --
                        for h in range(2):
                            for c in range(nch):
                                jlo = max(0, c - 4 * qb)   # first valid q strip
                                if jlo > 0:
                                    nc.gpsimd.memset(
                                        pt_t[h][:, c, 0:jlo * 128], 0.0)
                                if jlo > 3:
                                    continue
                                tps = ps2.tile([128, 512], bf16, tag="tp",
                                               bufs=2, name=f"tp{h}")
                                for j in range(jlo, 4):
                                    nc.tensor.matmul(
                                        tps[:, bass.ts(j, 128)],
                                        p_t[h][:, j, bass.ts(c, 128)],
                                        identb[:], is_transpose=True,
                                        start=(j == jlo), stop=(j == 3))
                                cp = nc.vector.tensor_copy if (c % 3 == 0) else nc.scalar.copy
                                cp(pt_t[h][:, c, bass.ds(jlo * 128, (4 - jlo) * 128)],
                                   tps[:, bass.ds(jlo * 128, (4 - jlo) * 128)])
                        # -- PV + normalize --
                        for h in range(2):
                            hg = hp * 2 + h
                            pv = ps2.tile([65, 512], f32, tag=f"pv{h}")
                            for c in range(nch):
                                nc.tensor.matmul(pv[:], va[kv][:, c, :], pt_t[h][:, c, :],
                                                 start=(c == 0), stop=(c == nch - 1))
                            rr = stp.tile([1, 512], f32, tag=f"rr{h}")
                            nc.vector.reciprocal(rr[:], pv[64:65, :])
                            nc.sync.dma_start(scr_d[hg, qb, :], rr[:])
                            bc = stp.tile([64, 512], f32, tag=f"bc{h}")
                            nc.sync.dma_start(
                                bc[:], scr_d[hg:hg + 1, qb, :].to_broadcast((64, 512)))
                            nc.vector.tensor_mul(
                                ot[hp][bass.ds(h * 64, 64), bass.ts(qb, 512)],
                                pv[0:64, :], bc[:])
                if DEBUG and hp == 0:
                    dt1 = ap.tile([128, 4, S], f32, tag="dbg3", name="cpp")
                    nc.vector.tensor_copy(dt1[:], p_t[0][:])
                    nc.sync.dma_start(dbg["d_p"][:], dt1[:, :, :].rearrange("p a b -> p (a b)"))
                    dt2 = ap.tile([128, NDC, SB], f32, tag="dbg4", name="cppt")
                    nc.vector.tensor_copy(dt2[:], pt_t[0][:])
                    nc.sync.dma_start(dbg["d_pt"][:], dt2[:, :, :].rearrange("p a b -> p (a b)"))

            if DEBUG:
                with tc.tile_pool(name="dbgq", bufs=1) as dq:
                    dt3 = dq.tile([128, S], f32, tag="dbg5", name="cpot")
                    nc.vector.tensor_copy(dt3[:], ot[0][:])
                    nc.sync.dma_start(dbg["d_ot"][:], dt3[:])

            # ---------------- phase 3: output projection ----------------
            with tc.tile_pool(name="wop", bufs=1) as wp, \
                 tc.tile_pool(name="op", bufs=3) as op, \
                 tc.tile_pool(name="ps3", bufs=2, space="PSUM") as ps3:
                wo_sb = wp.tile([128, 4, DIM], f32r)
                for fc in range(4):
                    nc.sync.dma_start(wo_sb[:, fc, :], wo_d[fc * 128:(fc + 1) * 128, :])
                for m in range(16):
                    for sb_i in range(NSB):
                        ps_o = ps3.tile([128, SB], f32, tag="pso")
                        for fc in range(4):
                            nc.tensor.matmul(
                                ps_o[:], wo_sb[:, fc, bass.ts(m, 128)],
                                ot[fc][:, bass.ts(sb_i, SB)],
                                start=(fc == 0), stop=(fc == 3))
                        osb = op.tile([128, SB], f32, tag="osb")
                        nc.scalar.copy(osb[:], ps_o[:])
                        nc.sync.dma_start(
                            oT_d[bass.ts(m, 128), bass.ts(sb_i, SB)], osb[:])

    nc.compile()
    return nc


_PROG = None


def kernel(x, wq, wk, wv, wo):
    global _PROG
    if _PROG is None:
        _PROG = build_program()
    nc = _PROG

    twq = _ternarize(wq) / 8.0          # fold softmax scale into q
    twk = _ternarize(wk)
    twv = _ternarize(wv)
    two = _ternarize(wo)
    tri_np = ((1.0 - np.tril(np.ones((128, 128)))) * -1e30).astype(bf)

    in_maps = []
    for c in range(8):
        b, hq = c % 2, c // 2
        xT = np.ascontiguousarray(x[b].astype(np.float32).T)      # [DIM, S]
        xhi, xlo = _split_hi_lo(xT)
        qcols = slice(hq * 512, (hq + 1) * 512)
        kvcols = slice(hq * 128, (hq + 1) * 128)
        in_maps.append({
            "xhi": xhi,
            "xlo": xlo,
            "wq": np.ascontiguousarray(twq.T[:, qcols]).astype(bf),
            "wk": np.ascontiguousarray(twk.T[:, kvcols]).astype(bf),
            "wv": np.ascontiguousarray(twv.T[:, kvcols]).astype(bf),
            "wo": np.ascontiguousarray(two.T[hq * 512:(hq + 1) * 512, :]).astype(np.float32),
            "tri": tri_np,
        })

    res = run_bass_kernel_spmd(nc, in_maps, list(range(8)))

    out = np.zeros((BSZ, SEQ, DIM), np.float32)
    for c in range(8):
        b = c % 2
        out[b] += res.results[c]["oT"].T
    return out



# revision 36
# speedup vs baseline: 1.1916x; 1.1916x over previous
import sys
sys.path.insert(0, '/opt/trn_rl_repo')
import math
import numpy as np
import ml_dtypes

import concourse.bass as bass
import concourse.tile as tile
from concourse import bacc, mybir
from concourse.bass_utils import run_bass_kernel_spmd
from concourse.masks import make_identity

DIM = 2048
BSZ, SEQ = 2, 2048
THRESHOLD = 0.05
S = SEQ
SB = 512
NSB = S // SB            # 4
NDC = DIM // 128         # 16
NQT = S // 128           # 16 q tiles per head
NM = 4                   # head pairs per core (head m & m+4 packed in partitions)
PTOT = 64 * NQT * (NQT + 1)   # compact causal P row length: sum 128*(qi+1)

f32 = mybir.dt.float32
f32r = mybir.dt.float32r
bf16 = mybir.dt.bfloat16
f8e4 = mybir.dt.float8e4
bf = ml_dtypes.bfloat16
f8 = ml_dtypes.float8_e4m3fn
DR = mybir.MatmulPerfMode.DoubleRow
EXP = mybir.ActivationFunctionType.Exp
AX = mybir.AxisListType.X
MAX = mybir.AluOpType.max
MIN = mybir.AluOpType.min
ADD = mybir.AluOpType.add
MUL = mybir.AluOpType.mult


def _ternarize(w):
    w = w.astype(np.float64)
    scale = max(np.abs(w).mean(), 1e-6)
    return np.where(w > THRESHOLD * scale, 1.0,
                    np.where(w < -THRESHOLD * scale, -1.0, 0.0))


def _poff(qi):
    # offset of row-block qi inside compact causal P buffer
    return 64 * qi * (qi + 1)


def build_program():
    nc = bacc.Bacc(None, target_bir_lowering=False, debug=False)

    def din(name, shape, dt):
        return nc.dram_tensor(name, list(shape), dt, kind="ExternalInput").ap()

    x_d = din("x", (DIM, S), f32r)        # x[b].T
    wq_d = din("wq", (DIM, 512), f32r)    # ternary(wq).T/8, head-pair col order
    wk_d = din("wk", (DIM, 128), f32r)    # [kv0|kv1]
    wv_d = din("wv", (DIM, 128), f32r)
    wo_d = din("wo", (128, 4, 2, DIM), f8e4)  # [part, fc, limb(w, w/16), col]
    tri_d = din("tri", (128, 128), f32)   # 0 lower/diag, -1e30 above diag
    oT_d = nc.dram_tensor("oT", [DIM, S], bf16, kind="ExternalOutput").ap()
    rr_d = nc.dram_tensor("rr", [8, NSB, SB], f32).ap()   # recip denominators

    with tile.TileContext(nc) as tc:
        with tc.tile_pool(name="persist", bufs=1) as pp:
            qt = [pp.tile([128, S], f32r, tag=f"qt{m}", name=f"qt{m}") for m in range(NM)]
            kt = pp.tile([128, S], f32r, name="kt")
            va = pp.tile([128, NDC, 130], bf16, name="va")   # per chunk: [kv0 f64|ones|kv1 f64|ones]
            ot = [pp.tile([128, S], bf16, tag=f"ot{m}", name=f"ot{m}") for m in range(NM)]
            ot8 = pp.tile([128, 4, 2, S], f8e4, name="ot8")
            tri = pp.tile([128, 128], f32, name="tri")
            nc.sync.dma_start(tri[:], tri_d[:])
            identb = pp.tile([128, 128], bf16, name="identb")
            make_identity(nc, identb[:])
            identf = pp.tile([128, 128], f32, name="identf")
            make_identity(nc, identf[:])
            nc.vector.memset(va[:, :, 64:65], 1.0)
            nc.vector.memset(va[:, :, 129:130], 1.0)

            # ---------------- phase 1: projections (fp32r) ----------------
            with tc.tile_pool(name="w1", bufs=1) as wp, \
                 tc.tile_pool(name="xp", bufs=4) as xp, \
                 tc.tile_pool(name="ev1", bufs=2) as ev, \
                 tc.tile_pool(name="ps1", bufs=1, space="PSUM") as psp:
                wq_sb = wp.tile([128, NDC, 512], f32r, name="wq_sb")
                wk_sb = wp.tile([128, NDC, 128], f32r, name="wk_sb")
                wv_sb = wp.tile([128, NDC, 128], f32r, name="wv_sb")
                for sb_i in range(NSB):
                    ssl = bass.ts(sb_i, SB)
                    ps_q = [psp.tile([128, SB], f32, tag=f"psq{m}", name=f"psq{m}")
                            for m in range(NM)]
                    ps_k = psp.tile([128, SB], f32, tag="psk")
                    ps_v = psp.tile([128, SB], f32, tag="psv")
                    for dc in range(NDC):
                        xc = xp.tile([128, SB], f32r, tag="xc")
                        nc.sync.dma_start(xc[:], x_d[dc * 128:(dc + 1) * 128, ssl])
                        if sb_i == 0 and dc == 0:
                            nc.sync.dma_start(
                                wq_sb[:], wq_d.rearrange("(a p) b -> p a b", p=128))
                            nc.sync.dma_start(
                                wk_sb[:], wk_d.rearrange("(a p) b -> p a b", p=128))
                            nc.sync.dma_start(
                                wv_sb[:], wv_d.rearrange("(a p) b -> p a b", p=128))
                        st = (dc == 0)
                        sp = (dc == NDC - 1)
                        for m in range(NM):
                            nc.tensor.matmul(ps_q[m][:], wq_sb[:, dc, bass.ts(m, 128)],
                                             xc[:], start=st, stop=sp)
                        nc.tensor.matmul(ps_k[:], wk_sb[:, dc, :], xc[:], start=st, stop=sp)
                        nc.tensor.matmul(ps_v[:], wv_sb[:, dc, :], xc[:], start=st, stop=sp)
                    # evacuate
                    for m in range(NM):
                        nc.vector.tensor_copy(qt[m][:, ssl], ps_q[m][:])
                    nc.vector.tensor_copy(kt[:, ssl], ps_k[:])
                    vts = ev.tile([128, SB], f32, tag="vts")
                    nc.vector.tensor_copy(vts[:], ps_v[:])
                    # transpose V^T[f, s-chunk] -> V[s, f] per 128-block
                    ps_t = psp.tile([128, 4, 128], f32, tag="pst")
                    for j in range(4):
                        nc.tensor.matmul(ps_t[:, j, :], vts[:, bass.ts(j, 128)],
                                         identf[:], is_transpose=True,
                                         start=True, stop=True)
                    for j in range(4):
                        ch = sb_i * 4 + j
                        nc.vector.tensor_copy(va[:, ch, 0:64], ps_t[:, j, 0:64])
                        nc.vector.tensor_copy(va[:, ch, 65:129], ps_t[:, j, 64:128])

            # ---------------- phase 2: attention ----------------
            with tc.tile_pool(name="att", bufs=1) as ap, \
                 tc.tile_pool(name="stat", bufs=3) as stp, \
                 tc.tile_pool(name="nrm", bufs=1) as nrm, \
                 tc.tile_pool(name="ps2", bufs=1, space="PSUM") as ps2:
                p_t = [ap.tile([128, PTOT], bf16, tag=f"p{i}", name=f"p{i}") for i in range(2)]
                pt_t = [ap.tile([128, NDC, SB], bf16, tag=f"pt{i}", name=f"pt{i}")
                        for i in range(2)]

                # tp-work queue: list of closures from previous stage
                pending = []

                def emit_some(k):
                    for _ in range(min(k, len(pending))):
                        pending.pop(0)()

                def make_tp_work(m, h, pbuf):
                    """DMA-transpose + PV closures for stage (m, h); the
                    1/denominator normalize is deferred to a per-stage batch."""
                    units = []
                    hg = m + 4 * h

                    def tp_chunk(qb, c):
                        def run():
                            ptb = pt_t[qb % 2]
                            jlo = max(0, c - 4 * qb)
                            tps = ps2.tile([128, SB], bf16, tag="tps", bufs=1,
                                           name="tps")
                            for j in range(jlo, 4):
                                qj = 4 * qb + j
                                nc.tensor.matmul(
                                    tps[:, bass.ts(j, 128)],
                                    pbuf[:, bass.ds(_poff(qj) + 128 * c, 128)],
                                    identb[:], is_transpose=True,
                                    start=(j == jlo), stop=(j == 3))
                            if jlo > 0:
                                nc.gpsimd.memset(ptb[:, c, 0:jlo * 128], 0.0)
                            cp = nc.vector.tensor_copy if c % 2 else nc.scalar.copy
                            cp(ptb[:, c, bass.ds(jlo * 128, (4 - jlo) * 128)],
                               tps[:, bass.ds(jlo * 128, (4 - jlo) * 128)])
                        return run

                    def pv_qb(qb):
                        def run():
                            nch = 4 * (qb + 1)
                            ptb = pt_t[qb % 2]
                            pv = ps2.tile([65, SB], f32, tag="pv", name="pv")
                            for c in range(nch):
                                nc.tensor.matmul(pv[:], va[:, c, bass.ds(65 * h, 65)],
                                                 ptb[:, c, :],
                                                 start=(c == 0), stop=(c == nch - 1))
                            rr = stp.tile([1, SB], f32, tag="rr", name="rr")
                            nc.vector.reciprocal(rr[:], pv[64:65, :])
                            nc.sync.dma_start(rr_d[hg, qb, :], rr[:])
                            if qb % 2 == 0:
                                nc.scalar.copy(
                                    ot[m][bass.ds(64 * h, 64), bass.ts(qb, SB)],
                                    pv[0:64, :])
                            else:
                                nc.vector.tensor_copy(
                                    ot[m][bass.ds(64 * h, 64), bass.ts(qb, SB)],
                                    pv[0:64, :])
                        return run

                    def norm_stage():
                        def run():
                            hsl = bass.ds(64 * h, 64)
                            rrb = nrm.tile([128, S], f32, tag="rrb", name="rrb")
                            nc.sync.dma_start(
                                rrb[hsl, :],
                                rr_d[hg:hg + 1, :, :].rearrange("a b c -> a (b c)")
                                .to_broadcast((64, S)))
                            # normalize + split into two fp8 limbs (on Pool)
                            nc.gpsimd.tensor_mul(ot[m][hsl, :], ot[m][hsl, :],
                                                 rrb[hsl, :])
                            nc.gpsimd.tensor_copy(ot8[hsl, m, 0, :], ot[m][hsl, :])
                            nc.gpsimd.tensor_sub(ot8[hsl, m, 1, :], ot[m][hsl, :],
                                                 ot8[hsl, m, 0, :])
                        return run

                    for qb in range(NSB):
                        for c in range(4 * (qb + 1)):
                            units.append(tp_chunk(qb, c))
                        units.append(pv_qb(qb))
                    units.append(norm_stage())
                    return units

                for stage in range(8):
                    m, h = stage % 4, stage // 4
                    hs = bass.ds(64 * h, 64)
                    pbuf = p_t[stage % 2]
                    nmx_p = [None] * NQT   # negmax [128,1] per row
                    for qi in range(NQT):
                        # ---- pass 1: row maxes; diag chunk exp'd in place ----
                        kw = 128 * (qi + 1)
                        nk = (kw + 511) // 512
                        nmx = stp.tile([128, 4], f32, tag="nmx", name="nmx")
                        s1_last = None
                        nhalf = (nk + 1) // 2
                        for hf in range(nhalf):
                            ck = min(2, nk - 2 * hf)         # chunks in this tile
                            w = min(1024, kw - 1024 * hf)
                            s1 = ps2.tile([128, 2, SB], f32, tag="s1", bufs=2,
                                          name="s1")
                            for c2 in range(ck):
                                cw = min(512, w - 512 * c2)
                                nc.tensor.matmul(
                                    s1[:, c2, 0:cw],
                                    qt[m][hs, bass.ts(qi, 128)],
                                    kt[hs, bass.ds(1024 * hf + 512 * c2, cw)],
                                    start=True, stop=True)
                            if hf == nhalf - 1:
                                cl, cwl = ck - 1, w - 512 * (ck - 1)
                                nc.vector.tensor_add(
                                    s1[:, cl, cwl - 128:cwl],
                                    s1[:, cl, cwl - 128:cwl], tri[:])
                                s1_last = (s1, cl, cwl)
                            # one reduce covers both chunks when full
                            if w == 1024:
                                nc.vector.tensor_reduce(
                                    nmx[:, 2 * hf:2 * hf + 2], s1[:, :, :],
                                    AX, MAX, negate=True)
                            else:
                                nc.vector.tensor_reduce(
                                    nmx[:, 2 * hf:2 * hf + 1],
                                    s1[:, 0, 0:w] if ck == 1 else s1[:, 1, 0:w - 512],
                                    AX, MAX, negate=True)
                                if ck == 2:
                                    nc.vector.tensor_reduce(
                                        nmx[:, 2 * hf + 1:2 * hf + 2],
                                        s1[:, 0, 0:512], AX, MAX, negate=True)
                        ngm = stp.tile([128, 1], f32, tag="ngm", name="ngm")
                        nc.vector.tensor_reduce(ngm[:], nmx[:, 0:nk], AX, MIN)
                        nmx_p[qi] = ngm
                        # diagonal (last) chunk: exp directly from pass-1 psum
                        s1, cl, cwl = s1_last
                        nc.scalar.activation(
                            pbuf[:, bass.ds(_poff(qi) + 512 * (nk - 1), cwl)],
                            s1[:, cl, 0:cwl], EXP, bias=ngm[:], scale=1.0)
                        # ---- pass 2 (non-diagonal chunks) for previous row ----
                        if qi >= 1:
                            emit_pass2(nc, ps2, qt, kt, p_t, nmx_p, m, h, qi - 1,
                                       stage)
                        emit_some(3)
                    emit_pass2(nc, ps2, qt, kt, p_t, nmx_p, m, h, NQT - 1, stage)
                    emit_some(4)
                    pending.extend(make_tp_work(m, h, pbuf))
                    if stage == 7:
                        emit_some(10 ** 6)

            # ---------------- phase 3: output projection ----------------
            with tc.tile_pool(name="wop", bufs=1) as wp3, \
                 tc.tile_pool(name="op", bufs=4) as op, \
                 tc.tile_pool(name="ps3", bufs=3, space="PSUM") as ps3:
                wo_sb = wp3.tile([128, 4, 2, DIM], f8e4, name="wo_sb")
                nc.sync.dma_start(wo_sb[:], wo_d[:])
                for mo in range(16):
                    for sb_i in range(NSB):
                        ps_o = ps3.tile([128, SB], f32, tag="pso")
                        for fc in range(4):
                            nc.tensor.matmul(
                                ps_o[:], wo_sb[:, fc, :, bass.ts(mo, 128)],
                                ot8[:, fc, :, bass.ts(sb_i, SB)],
                                start=(fc == 0), stop=(fc == 3),
                                perf_mode=DR)
                        osb = op.tile([128, SB], bf16, tag="osb")
                        if (mo * NSB + sb_i) % 2 == 0:
                            nc.scalar.copy(osb[:], ps_o[:])
                        else:
                            nc.vector.tensor_copy(osb[:], ps_o[:])
                        nc.scalar.dma_start(
                            oT_d[bass.ts(mo, 128), bass.ts(sb_i, SB)], osb[:])

    nc.compile()
    return nc


def emit_pass2(nc, ps2, qt, kt, p_t, nmx_p, m, h, qi, stage):
    """Recompute the full 512-wide (non-diagonal) score chunks for row qi
    and exp them into p. The diagonal chunk was exp'd from pass-1 psum."""
    hs = bass.ds(64 * h, 64)
    pbuf = p_t[stage % 2]
    kw = 128 * (qi + 1)
    nfull = (kw - 1) // 512          # chunks before the diagonal one
    if nfull == 0:
        return
    ngm = nmx_p[qi]
    for half in range(0, 512 * nfull, 1024):
        hw_ = min(1024, 512 * nfull - half)
        s2 = ps2.tile([128, 1024], f32, tag="s2", bufs=1, name="s2")
        for c0 in range(0, hw_, 512):
            nc.tensor.matmul(s2[:, c0:c0 + 512],
                             qt[m][hs, bass.ts(qi, 128)],
                             kt[hs, bass.ds(half + c0, 512)],
                             start=True, stop=True)
        nc.scalar.activation(pbuf[:, bass.ds(_poff(qi) + half, hw_)],
                             s2[:, 0:hw_], EXP, bias=ngm[:], scale=1.0)


_PROG = None


def kernel(x, wq, wk, wv, wo):
    global _PROG
    if _PROG is None:
        _PROG = build_program()
    nc = _PROG

    twq = _ternarize(wq) / 8.0          # fold softmax scale into q
    twk = _ternarize(wk)
    twv = _ternarize(wv)
    two = _ternarize(wo)
    tri_np = ((1.0 - np.tril(np.ones((128, 128)))) * -1e30).astype(np.float32)

    # head-pair permutation: pair m holds heads (m, m+4) of the local group
    perm = []
    for m in range(4):
        perm += list(range(64 * m, 64 * m + 64))
        perm += list(range(64 * (m + 4), 64 * (m + 4) + 64))

    in_maps = []
    for c in range(8):
        b, hq = c % 2, c // 2
        xT = np.ascontiguousarray(x[b].astype(np.float32).T)      # [DIM, S]
        gq = slice(hq * 512, (hq + 1) * 512)
        gkv = slice(hq * 128, (hq + 1) * 128)
        wq_cols = twq.T[:, gq][:, perm]                            # [DIM, 512]
        wo_rows = two.T[gq, :][perm, :]                            # [512, DIM]
        wo8 = np.zeros((128, 4, 2, DIM), np.float32)
        for fc in range(4):
            wo8[:, fc, 0, :] = wo_rows[fc * 128:(fc + 1) * 128]
            wo8[:, fc, 1, :] = wo_rows[fc * 128:(fc + 1) * 128]
        in_maps.append({
            "x": xT,
            "wq": np.ascontiguousarray(wq_cols).astype(np.float32),
            "wk": np.ascontiguousarray(twk.T[:, gkv]).astype(np.float32),
            "wv": np.ascontiguousarray(twv.T[:, gkv]).astype(np.float32),
            "wo": wo8.astype(f8),
            "tri": tri_np,
        })

    res = run_bass_kernel_spmd(nc, in_maps, list(range(8)))

    out = np.zeros((BSZ, SEQ, DIM), np.float32)
    for c in range(8):
        b = c % 2
        out[b] += res.results[c]["oT"].astype(np.float32).T
    return out
